# revision 2
# baseline (speedup 1.0000x reference)
# Emu3 VQVAE vector-quantizer kernel for 8x TRN2 NeuronCores (Bass/Tile).
#
# Problem: hidden_state (8,1,256,32,32) f32, codebook (16384,256) f32
#   -> nearest-codebook-entry indices (8,1,32,32) int32
#   distances = |x|^2 + |e|^2 - 2 x.e ; argmin over K with first-index ties.
#
# Numerics: |e|^2 ~ 3e-7 while |x|^2 ~ 256, so in fp32 (xsq + esq) == xsq
# bitwise (esq < half-ulp always). The reference distances are therefore
# d = fl(xsq - fl(2*mm)) exactly, and ~4% of rows have exact fp32 ties at
# the min, so we must reproduce the quantized d values and first-index
# tie-breaking, not just argmax of the raw matmul.
#
# Sharding: data-parallel over the 8 batch entries (1024 tokens each);
# codebook replicated. Per core: fp32 PE matmul (tokens x codes, PSUM
# accumulates over C=256 in two 128-row passes, codebook pre-scaled by 2 so
# PSUM holds 2*mm exactly), ACT computes nd = 2*mm - xsq into an SBUF slab,
# DVE Max/MaxIndex give the argmax of nd (= argmin of d) with first-index
# tie semantics.

import numpy as np

B, T, C, H, W = 8, 1, 256, 32, 32
K = 16384
NCORES = 8
NTOK = H * W          # tokens per core
NTILES = NTOK // 128  # token tiles per core
KHALF = K // 2
CHUNK = 512
NSEC = 8              # codebook DMA sections
SEC = K // NSEC

_CACHE = {}


def _build_bass(matmul_dtype_name="float32"):
    from contextlib import ExitStack

    import concourse.bass as bass  # noqa: F401
    import concourse.mybir as mybir
    import concourse.tile as tile
    from concourse import bacc

    f32 = mybir.dt.float32
    mm_dt = getattr(mybir.dt, matmul_dtype_name)
    i32 = mybir.dt.int32
    u32 = mybir.dt.uint32
    AF = mybir.ActivationFunctionType
    ALU = mybir.AluOpType

    nc = bacc.Bacc(
        "TRN2",
        target_bir_lowering=False,
        debug=False,
        enable_asserts=False,
        num_devices=NCORES,
    )

    xT_d = nc.dram_tensor("xT", (2, 128, NTOK), mm_dt, kind="ExternalInput").ap()
    cb_d = nc.dram_tensor("cbT2", (2, 128, K), mm_dt, kind="ExternalInput").ap()
    nxsq_d = nc.dram_tensor("nxsq", (128, NTILES), f32, kind="ExternalInput").ap()
    out_d = nc.dram_tensor("idx", (NTILES, 128, 1), i32, kind="ExternalOutput").ap()

    with tile.TileContext(nc) as tc:
        with ExitStack() as ctx:
            cbp = ctx.enter_context(tc.tile_pool(name="cb", bufs=1))
            xp = ctx.enter_context(tc.tile_pool(name="x", bufs=1))
            sp = ctx.enter_context(tc.tile_pool(name="slab", bufs=2))
            pp = ctx.enter_context(tc.tile_pool(name="psum", bufs=8, space="PSUM"))
            smp = ctx.enter_context(tc.tile_pool(name="small", bufs=4))
            outp = ctx.enter_context(tc.tile_pool(name="outs", bufs=4))

            xts = []
            for cs in range(2):
                xt = xp.tile([128, NTOK], mm_dt, tag=f"x{cs}")
                nc.sync.dma_start(xt[:], xT_d[cs])
                xts.append(xt)
            nxsq = xp.tile([128, NTILES], f32, tag="nxsq")
            nc.sync.dma_start(nxsq[:], nxsq_d[:])

            cbs = [[None] * NSEC for _ in range(2)]
            for s in range(NSEC):
                for cs in range(2):
                    cbt = cbp.tile([128, SEC], mm_dt, tag=f"cb{cs}_{s}")
                    nc.sync.dma_start(cbt[:], cb_d[cs][:, s * SEC : (s + 1) * SEC])
                    cbs[cs][s] = cbt

            c8192 = xp.tile([128, 1], u32, tag="c8192")
            nc.vector.memset(c8192[:], KHALF)

            for t in range(NTILES):
                tops = []
                idxs = []
                for h in range(2):
                    slab = sp.tile([128, KHALF], f32, tag="slab")
                    for c in range(KHALF // CHUNK):
                        k0 = h * KHALF + c * CHUNK
                        s, off = divmod(k0, SEC)
                        ps = pp.tile([128, CHUNK], f32, tag="ps")
                        nc.tensor.matmul(
                            ps[:],
                            xts[0][:, t * 128 : (t + 1) * 128],
                            cbs[0][s][:, off : off + CHUNK],
                            start=True,
                            stop=False,
                        )
                        nc.tensor.matmul(
                            ps[:],
                            xts[1][:, t * 128 : (t + 1) * 128],
                            cbs[1][s][:, off : off + CHUNK],
                            start=False,
                            stop=True,
                        )
                        # nd = 2*mm - xsq  (PSUM already holds 2*mm)
                        nc.scalar.activation(
                            slab[:, c * CHUNK : (c + 1) * CHUNK],
                            ps[:],
                            AF.Identity,
                            bias=nxsq[:, t : t + 1],
                            scale=1.0,
                        )
                    top8 = smp.tile([128, 8], f32, tag="top8")
                    nc.vector.max(top8[:], slab[:])
                    idx8 = smp.tile([128, 8], u32, tag="idx8")
                    nc.vector.max_index(idx8[:], top8[:], slab[:])
                    tops.append(top8)
                    idxs.append(idx8)

                # combine halves; strict > so half 0 wins ties (first index)
                mask = smp.tile([128, 1], i32, tag="mask")
                nc.vector.tensor_tensor(
                    mask[:], tops[1][:, 0:1], tops[0][:, 0:1], op=ALU.is_gt
                )
                idx1p = smp.tile([128, 1], u32, tag="idx1p")
                nc.vector.tensor_tensor(
                    idx1p[:], idxs[1][:, 0:1], c8192[:], op=ALU.add
                )
                win = outp.tile([128, 1], u32, tag="win")
                nc.vector.tensor_copy(win[:], idxs[0][:, 0:1])
                nc.vector.copy_predicated(win[:], mask[:], idx1p[:])
                winI = outp.tile([128, 1], i32, tag="winI")
                nc.vector.tensor_copy(winI[:], win[:])
                nc.sync.dma_start(out_d[t], winI[:])

    nc.compile()
    return nc


def get_nc(matmul_dtype_name="float32"):
    key = ("nc", matmul_dtype_name)
    if key not in _CACHE:
        _CACHE[key] = _build_bass(matmul_dtype_name)
    return _CACHE[key]


def prepare_inputs(hidden_state, codebook):
    """Host-side shard prep: returns in_maps (list of 8 dicts)."""
    hs = np.ascontiguousarray(np.asarray(hidden_state, dtype=np.float32))
    cb = np.ascontiguousarray(np.asarray(codebook, dtype=np.float32))
    # per-core x^T: (C, H*W) is exactly hidden_state[b, 0] flattened
    xT = hs.reshape(B, C, NTOK)
    cb2 = np.ascontiguousarray((2.0 * cb.T).astype(np.float32).reshape(2, 128, K))
    in_maps = []
    for b in range(B):
        xb = np.ascontiguousarray(xT[b].reshape(2, 128, NTOK))
        xsq = np.sum(xT[b] * xT[b], axis=0, dtype=np.float32)  # (NTOK,)
        nxsq = np.ascontiguousarray((-xsq).reshape(NTILES, 128).T)  # (128, NTILES)
        in_maps.append({"xT": xb, "cbT2": cb2, "nxsq": nxsq})
    return in_maps


def kernel(hidden_state, codebook):
    from concourse.bass_utils import run_bass_kernel_spmd

    nc = get_nc()
    in_maps = prepare_inputs(hidden_state, codebook)
    res = run_bass_kernel_spmd(nc, in_maps, core_ids=list(range(NCORES)))
    out = np.stack(
        [res.results[b]["idx"].reshape(NTOK) for b in range(B)], axis=0
    ).astype(np.int32)
    return out.reshape(B, T, H, W)


# revision 22
# speedup vs baseline: 4.2840x; 4.2840x over previous
# Emu3 VQVAE vector-quantizer kernel for 8x TRN2 NeuronCores (Bass/Tile).
#
# Problem: hidden_state (8,1,256,32,32) f32, codebook (16384,256) f32
#   -> nearest-codebook-entry indices (8,1,32,32) int32
#   distances = |x|^2 + |e|^2 - 2 x.e ; argmin over K with first-index ties.
#
# Numerics: |e|^2 ~ 3e-7 while |x|^2 ~ 256, so in fp32 (xsq + esq) == xsq
# bitwise (esq < half-ulp always). The reference distances are therefore
# d = fl(xsq - fl(2*mm)) exactly, and ~4% of rows have exact fp32 ties at
# the min, so we must reproduce the quantized d values and first-index
# tie-breaking, not just argmax of the raw matmul.
#
# Sharding: data-parallel over the 8 batch entries (1024 tokens each);
# codebook replicated.
#
# Per core: PE matmul (tokens x codes; PSUM accumulates over C=256; codebook
# pre-scaled by 2 so PSUM holds 2*mm exactly). Matmul dtype modes:
#   float32  - exact, 4 cycles/row
#   bf16x3   - exact (hi/lo bf16 split, 3 terms, products exact in fp32,
#              residual xl*el ~2^-17 relative: no observed index changes),
#              6 passes at 1 cycle/row
#   float32r - tf32-like reduced precision, ~5/8192 flipped indices
#
# Argmin: ACT computes d = fl(xsq - 2mm) (the reference's quantization).
# Distances of one token span < 2^13 fp32 ulps (Cauchy-Schwarz bound,
# host-verified), so key = (d - base)*S + k packs (distance, index) into an
# exact fp32 integer < 2^24 for 2048-wide sections. One fused
# tensor_tensor_reduce per section gives min-key = lexicographic
# (d, k)-min = first-index argmin. Tiny decode ops combine 8 sections.

import numpy as np

B, T, C, H, W = 8, 1, 256, 32, 32
K = 16384
NCORES = 8
NTOK = H * W          # tokens per core
NTILES = NTOK // 128  # token tiles per core
CHUNK = 512
SECW = 2048           # argmin section width (11 index bits)
NSECT = K // SECW     # 8
NSEC = 8              # codebook DMA sections
SEC = K // NSEC

_CACHE = {}


def _build_bass(matmul_dtype_name="float32", repeats=1, ablate="full", keys_on="dve"):
    from contextlib import ExitStack

    import concourse.bass as bass  # noqa: F401
    import concourse.mybir as mybir
    import concourse.tile as tile
    from concourse import bacc

    f32 = mybir.dt.float32
    bf16 = mybir.dt.bfloat16
    is_bf16x3 = matmul_dtype_name == "bf16x3"
    mm_dt = bf16 if is_bf16x3 else getattr(mybir.dt, matmul_dtype_name)
    i32 = mybir.dt.int32
    AF = mybir.ActivationFunctionType
    ALU = mybir.AluOpType

    nc = bacc.Bacc(
        "TRN2",
        target_bir_lowering=False,
        debug=False,
        enable_asserts=False,
        num_devices=NCORES,
    )

    # NS: hi/lo bf16 split factor (1 for plain fp32/fp32r)
    NS = 2 if is_bf16x3 else 1
    xT_d = nc.dram_tensor("xT", (NS, 2, 128, NTOK), mm_dt, kind="ExternalInput").ap()
    cb_d = nc.dram_tensor("cbT2", (NS, 2, 128, K), mm_dt, kind="ExternalInput").ap()
    xsq_d = nc.dram_tensor("xsqp", (128, NTILES), f32, kind="ExternalInput").ap()
    base_d = nc.dram_tensor("base", (128, NTILES), f32, kind="ExternalInput").ap()
    scal_d = nc.dram_tensor("scal", (128, NTILES), f32, kind="ExternalInput").ap()
    nbs_d = nc.dram_tensor("nbaseS", (128, NTILES), f32, kind="ExternalInput").ap()
    iot_d = nc.dram_tensor(
        "iotas", (128, SECW + 2 * NSECT), f32, kind="ExternalInput"
    ).ap()
    out_d = nc.dram_tensor("idx", (NTILES, 128, 1), i32, kind="ExternalOutput").ap()

    with tile.TileContext(nc) as tc:
        with ExitStack() as ctx:
            cbp = ctx.enter_context(tc.tile_pool(name="cb", bufs=1))
            xp = ctx.enter_context(tc.tile_pool(name="x", bufs=1))
            sp = ctx.enter_context(tc.tile_pool(name="slab", bufs=3))
            pp = ctx.enter_context(tc.tile_pool(name="psum", bufs=8, space="PSUM"))
            smp = ctx.enter_context(tc.tile_pool(name="small", bufs=4))
            outp = ctx.enter_context(tc.tile_pool(name="outs", bufs=4))

            xts = {}
            for hl in range(NS):
                for cs in range(2):
                    xt = xp.tile([128, NTOK], mm_dt, tag=f"x{hl}_{cs}")
                    nc.sync.dma_start(xt[:], xT_d[hl][cs])
                    xts[hl, cs] = xt
            xsq = xp.tile([128, NTILES], f32, tag="xsq")
            nc.sync.dma_start(xsq[:], xsq_d[:])
            base = xp.tile([128, NTILES], f32, tag="base")
            nc.sync.dma_start(base[:], base_d[:])
            scal = xp.tile([128, NTILES], f32, tag="scal")
            nc.sync.dma_start(scal[:], scal_d[:])
            nbs = xp.tile([128, NTILES], f32, tag="nbs")
            nc.sync.dma_start(nbs[:], nbs_d[:])

            cbs = {}
            for s in range(NSEC):
                for hl in range(NS):
                    for cs in range(2):
                        cbt = cbp.tile([128, SEC], mm_dt, tag=f"cb{hl}_{cs}_{s}")
                        nc.sync.dma_start(
                            cbt[:], cb_d[hl][cs][:, s * SEC : (s + 1) * SEC]
                        )
                        cbs[hl, cs, s] = cbt

            # constant iota tiles (host-provided; gpsimd.iota crashes trn2 here)
            iotas = xp.tile([128, SECW + 2 * NSECT], f32, tag="iotas")
            nc.sync.dma_start(iotas[:], iot_d[:])
            iota_sec = iotas[:, 0:SECW]
            iota8 = iotas[:, SECW : SECW + NSECT]
            iota8w = iotas[:, SECW + NSECT : SECW + 2 * NSECT]
            # int consts for bitwise decode (tensor_tensor operands)
            c_klo = xp.tile([128, NSECT], i32, tag="c_klo")
            nc.vector.memset(c_klo[:], SECW - 1)
            c_khi = xp.tile([128, NSECT], i32, tag="c_khi")
            nc.vector.memset(c_khi[:], -SECW)  # 0xFFFFF800
            c_s = xp.tile([128, 1], i32, tag="c_s")
            nc.vector.memset(c_s[:], NSECT - 1)

            # matmul term order: accumulation passes over
            # (x hi/lo, cb hi/lo, C-half), dropping xl*el.
            if is_bf16x3:
                TERMS = [(0, 0, 0), (0, 0, 1), (0, 1, 0), (0, 1, 1), (1, 0, 0), (1, 0, 1)]
            else:
                TERMS = [(0, 0, 0), (0, 0, 1)]

            for t in [t for _ in range(repeats) for t in range(NTILES)]:
                minik = smp.tile([128, NSECT], f32, tag="minik")
                for sec in range(NSECT):
                    slab = sp.tile([128, SECW], f32, tag="slab")
                    pss = [
                        pp.tile([128, CHUNK], f32, tag="ps", name=f"ps_{t}_{sec}_{ci}")
                        for ci in range(SECW // CHUNK)
                    ]
                    for ti, (xhl, ehl, cs) in enumerate(TERMS):
                        for ci in range(SECW // CHUNK):
                            k0 = sec * SECW + ci * CHUNK
                            s, off = divmod(k0, SEC)
                            nc.tensor.matmul(
                                pss[ci][:],
                                xts[xhl, cs][:, t * 128 : (t + 1) * 128],
                                cbs[ehl, cs, s][:, off : off + CHUNK],
                                start=(ti == 0),
                                stop=(ti == len(TERMS) - 1),
                            )
                    for ci in range(SECW // CHUNK):
                        if ablate == "peonly":
                            nc.scalar.activation(
                                slab[:, ci : ci + 1],
                                pss[ci][:, 0:1],
                                AF.Identity,
                                bias=xsq[:, t : t + 1],
                                scale=-1.0,
                            )
                            continue
                        # d = fl(xsq - 2mm): the reference's quantized distance
                        nc.scalar.activation(
                            slab[:, ci * CHUNK : (ci + 1) * CHUNK],
                            pss[ci][:],
                            AF.Identity,
                            bias=xsq[:, t : t + 1],
                            scale=-1.0,
                        )
                    if ablate != "full":
                        nc.vector.tensor_copy(minik[:, sec : sec + 1], slab[:, 0:1])
                        continue
                    # keys = (d - base)*S + k_local  (exact pow2 scalings),
                    # then min-reduce -> minik[sec]
                    if keys_on in ("dve", "dve+pool"):
                        nc.vector.tensor_scalar(
                            slab[:],
                            slab[:],
                            base[:, t : t + 1],
                            scal[:, t : t + 1],
                            op0=ALU.subtract,
                            op1=ALU.mult,
                        )
                        if keys_on == "dve+pool":
                            nc.gpsimd.tensor_tensor(
                                slab[:], slab[:], iota_sec, op=ALU.add
                            )
                        else:
                            nc.vector.tensor_tensor(
                                slab[:], slab[:], iota_sec, op=ALU.add
                            )
                    else:
                        # keys0 = d*S - base*S on ACT (both pow2-exact)
                        nc.scalar.activation(
                            slab[:],
                            slab[:],
                            AF.Identity,
                            bias=nbs[:, t : t + 1],
                            scale=scal[:, t : t + 1],
                        )
                        if keys_on == "act+pool":
                            nc.gpsimd.tensor_tensor(
                                slab[:], slab[:], iota_sec, op=ALU.add
                            )
                        else:
                            nc.vector.tensor_tensor(
                                slab[:], slab[:], iota_sec, op=ALU.add
                            )
                    nc.vector.tensor_reduce(
                        minik[:, sec : sec + 1],
                        slab[:],
                        axis=mybir.AxisListType.X,
                        op=ALU.min,
                    )

                # decode: minik_s = dq_s*SECW + k_s (exact fp32 ints);
                # split via int bitwise ops (DVE mod/floor don't exist)
                minik_i = smp.tile([128, NSECT], i32, tag="minik_i")
                nc.vector.tensor_copy(minik_i[:], minik[:])
                kmod_i = smp.tile([128, NSECT], i32, tag="kmod_i")
                nc.vector.tensor_tensor(
                    kmod_i[:], minik_i[:], c_klo[:], op=ALU.bitwise_and
                )
                kmod = smp.tile([128, NSECT], f32, tag="kmod")
                nc.vector.tensor_copy(kmod[:], kmod_i[:])
                dqw_i = smp.tile([128, NSECT], i32, tag="dqw_i")
                nc.vector.tensor_tensor(
                    dqw_i[:], minik_i[:], c_khi[:], op=ALU.bitwise_and
                )
                dqw = smp.tile([128, NSECT], f32, tag="dqw")
                nc.vector.tensor_copy(dqw[:], dqw_i[:])
                # keys2 = dq_s*NSECT + s  (exact, < 2^17)
                keys2 = smp.tile([128, NSECT], f32, tag="keys2")
                nc.vector.tensor_scalar(
                    keys2[:], dqw[:], float(NSECT) / float(SECW), None, op0=ALU.mult
                )
                nc.vector.tensor_tensor(keys2[:], keys2[:], iota8, op=ALU.add)
                m2 = smp.tile([128, 1], f32, tag="m2")
                nc.vector.tensor_reduce(
                    m2[:], keys2[:], axis=mybir.AxisListType.X, op=ALU.min
                )
                m2i = smp.tile([128, 1], i32, tag="m2i")
                nc.vector.tensor_copy(m2i[:], m2[:])
                sstar_i = smp.tile([128, 1], i32, tag="sstar_i")
                nc.vector.tensor_tensor(
                    sstar_i[:], m2i[:], c_s[:], op=ALU.bitwise_and
                )
                sstar = smp.tile([128, 1], f32, tag="sstar")
                nc.vector.tensor_copy(sstar[:], sstar_i[:])
                # select kfull = s*SECW + k_s of the winning section
                mask8 = smp.tile([128, NSECT], f32, tag="mask8")
                nc.vector.tensor_scalar(
                    mask8[:], iota8, sstar[:], None, op0=ALU.is_equal
                )
                kfull = smp.tile([128, NSECT], f32, tag="kfull")
                nc.vector.tensor_tensor(kfull[:], iota8w, kmod[:], op=ALU.add)
                nc.vector.tensor_tensor(kfull[:], kfull[:], mask8[:], op=ALU.mult)
                kwin = outp.tile([128, 1], f32, tag="kwin")
                nc.vector.tensor_reduce(
                    kwin[:], kfull[:], axis=mybir.AxisListType.X, op=ALU.add
                )
                winI = outp.tile([128, 1], i32, tag="winI")
                nc.vector.tensor_copy(winI[:], kwin[:])
                nc.sync.dma_start(out_d[t], winI[:])

    nc.compile()
    return nc


def get_nc(matmul_dtype_name="float32", repeats=1, ablate="full", keys_on="dve"):
    key = ("nc", matmul_dtype_name, repeats, ablate, keys_on)
    if key not in _CACHE:
        _CACHE[key] = _build_bass(matmul_dtype_name, repeats, ablate, keys_on)
    return _CACHE[key]


def prepare_inputs(hidden_state, codebook, mode="float32"):
    """Host-side shard prep: returns in_maps (list of 8 dicts)."""
    import ml_dtypes

    hs = np.ascontiguousarray(np.asarray(hidden_state, dtype=np.float32))
    cb = np.ascontiguousarray(np.asarray(codebook, dtype=np.float32))
    # per-core x^T: (C, H*W) is exactly hidden_state[b, 0] flattened
    xT = hs.reshape(B, C, NTOK)
    cb2 = (2.0 * cb.T).astype(np.float32)  # (C, K), exact doubling
    if mode == "bf16x3":
        cb2h = cb2.astype(ml_dtypes.bfloat16)
        cb2l = (cb2 - cb2h.astype(np.float32)).astype(ml_dtypes.bfloat16)
        cb_in = np.ascontiguousarray(np.stack([cb2h, cb2l]).reshape(2, 2, 128, K))
    else:
        cb_in = np.ascontiguousarray(cb2.reshape(1, 2, 128, K))

    iota_row = np.concatenate(
        [
            np.arange(SECW, dtype=np.float32),
            np.arange(NSECT, dtype=np.float32),
            np.arange(NSECT, dtype=np.float32) * SECW,
        ]
    )
    iotas = np.ascontiguousarray(np.broadcast_to(iota_row, (128, iota_row.size)))

    # |2*e_k| bound for the per-token distance-spread budget
    emax = float(np.max(np.linalg.norm(2.0 * cb.astype(np.float64), axis=1)))

    in_maps = []
    for b in range(B):
        xb32 = xT[b]
        if mode == "bf16x3":
            xh = xb32.astype(ml_dtypes.bfloat16)
            xl = (xb32 - xh.astype(np.float32)).astype(ml_dtypes.bfloat16)
            xin = np.ascontiguousarray(np.stack([xh, xl]).reshape(2, 2, 128, NTOK))
        else:
            xin = np.ascontiguousarray(xb32.reshape(1, 2, 128, NTOK))
        xsq = np.sum(xb32 * xb32, axis=0, dtype=np.float32)  # (NTOK,)

        # base_t <= min_k d, and (d - base)/ulp(base) < 2^13 guaranteed:
        # |2mm| <= |x| * max|2e_k| (Cauchy-Schwarz), 20% margin
        xsq64 = xsq.astype(np.float64)
        bound = np.sqrt(xsq64) * emax * 1.2 + 1e-6
        base = (xsq64 - bound).astype(np.float32)
        # ulp of base's binade; d - base is always a multiple of this
        _, exp = np.frexp(base)
        ulp = np.ldexp(np.float64(1.0), exp - 24)
        dq_max = (xsq64 + bound - base.astype(np.float64)) / ulp
        assert (base > 0).all() and (dq_max < 8100).all(), (
            "distance-spread exceeds 13-bit key budget; "
            f"max dq={dq_max.max():.0f}"
        )
        scal = np.ldexp(np.float32(SECW), -(exp - 24)).astype(np.float32)  # SECW/ulp
        nbaseS = (-(base.astype(np.float64) * scal.astype(np.float64))).astype(
            np.float32
        )  # exact: base * pow2

        def pt(a):  # (NTOK,) -> (128, NTILES)
            return np.ascontiguousarray(a.reshape(NTILES, 128).T)

        in_maps.append(
            {
                "xT": xin,
                "cbT2": cb_in,
                "xsqp": pt(xsq),
                "base": pt(base),
                "scal": pt(scal),
                "nbaseS": pt(nbaseS),
                "iotas": iotas,
            }
        )
    return in_maps


MODE = "bf16x3"


def kernel(hidden_state, codebook):
    from concourse.bass_utils import run_bass_kernel_spmd

    nc = get_nc(MODE)
    in_maps = prepare_inputs(hidden_state, codebook, MODE)
    res = run_bass_kernel_spmd(nc, in_maps, core_ids=list(range(NCORES)))
    out = np.stack(
        [res.results[b]["idx"].reshape(NTOK) for b in range(B)], axis=0
    ).astype(np.int32)
    return out.reshape(B, T, H, W)
